# revision 10
# baseline (speedup 1.0000x reference)
"""Multi-head causal self-attention Trainium2 kernel (8 NeuronCores, SPMD).

Strategy (head-parallel + token-parallel out-proj):
  - Host pre-transposes x -> xT [D, T] (T = B*TB flattened tokens) and slices
    per-core qkv weight shards (2 heads/core, columns [q2|k2|v2] -> [D, 384]).
  - Each core computes qkvT = w_shard^T @ x  (feature-partitioned layout
    [384, T]), giving qT/kT/vT [128, T] directly (2 heads stacked, 64+64).
  - V is re-transposed on the PE (32 small 128x128 transposes) into natural
    [s, e] layout with an appended ones-column, so the PV matmul (M=65)
    accumulates the softmax denominator Z in PSUM row 64 for free.
  - Scores ST[s,t] = kT^T @ qT are computed transposed, 2 heads row-packed on
    the PE (K=64 each at array rows 0-63 / 64-127).  Causality is handled by
    shrinking the moving window (never computing fully-masked columns) plus a
    single constant 128x128 triangular mask on diagonal blocks.
  - exp via ACT with the 1/8 scale fused into the activation's scale operand;
    no running max is needed (scores ~ N(0,1), T=2048 -> max ~ 4-5).
  - OT[e,t] is normalized by 1/Z via a K=2 broadcast matmul + DVE multiply,
    then AllToAll redistributes head-shards -> token-shards, and each core
    computes its 512-token slice of the out-projection with the full w_out.
  - Host concatenates the 8 per-core [512, D] outputs.

All matmuls run as float32r (full PE rate at moving-dim >= 256, ~fp32 data).
"""

import os
import sys

import numpy as np

for _p in ("/opt/trn_rl_repo", "/root/.axon_site/_ro/trn_rl_repo"):
    if os.path.isdir(_p) and _p not in sys.path:
        sys.path.insert(0, _p)

import concourse.bass as bass  # noqa: E402
import concourse.mybir as mybir  # noqa: E402
import concourse.tile as tile  # noqa: E402
from concourse import bacc  # noqa: E402

F32 = mybir.dt.float32
F32R = mybir.dt.float32r
EXP = mybir.ActivationFunctionType.Exp

D = 1024         # model dim
DH = 64          # head dim
NCORES = 8
P = 128          # partitions
MW = 3           # m-tiles of the qkv shard: q(2 heads)|k|v, each 128 wide
SB = 128         # s-block (key block)
SCALE = DH ** -0.5

VA = 2 * DH + 2  # V_aug stride: [V_A(64) | ones | V_B(64) | ones] = 130


def build_nc(TB=2048, W=512):
    """Build the SPMD Bass program. TB = tokens per batch, W = token window."""
    T = 2 * TB                  # flattened tokens (2 batches)
    NW = TB // W                # windows per batch
    assert T % W == 0 and T // W == NCORES, "A2A shards must equal cores"
    KT = D // P                 # contraction tiles (8)
    NBLK = TB // SB             # s-blocks per batch
    YT = W // P                 # out-proj token tiles per core
    NMW = D // 512              # out-proj N windows

    nc = bacc.Bacc(
        "TRN2", target_bir_lowering=False, debug=False, num_devices=NCORES
    )

    xT = nc.dram_tensor("xT", [D, T], F32, kind="ExternalInput")
    w_sh = nc.dram_tensor("w_sh", [D, MW * P], F32, kind="ExternalInput")
    b_sh = nc.dram_tensor("b_sh", [P, MW], F32, kind="ExternalInput")
    w_out = nc.dram_tensor("w_out", [D, D], F32, kind="ExternalInput")
    b_ob = nc.dram_tensor("b_ob", [P, D], F32, kind="ExternalInput")
    y = nc.dram_tensor("y", [W, D], F32, kind="ExternalOutput")

    ident_c = nc.inline_tensor(np.eye(P, dtype=np.float32), "ident_c")
    tri_np = (np.arange(P)[:, None] <= np.arange(P)[None, :]).astype(np.float32)
    tri_c = nc.inline_tensor(tri_np, "tri_c")
    selA_np = np.zeros((1, P), np.float32)
    selA_np[0, :DH] = 1.0
    selA_c = nc.inline_tensor(selA_np, "selA_c")
    selB_np = np.zeros((1, P), np.float32)
    selB_np[0, DH:] = 1.0
    selB_c = nc.inline_tensor(selB_np, "selB_c")
    ones_c = nc.inline_tensor(np.ones((P, 1), np.float32), "ones_c")

    with tile.TileContext(nc) as tc:
        with (
            tc.tile_pool(name="consts", bufs=1) as cp,
            tc.tile_pool(name="wq", bufs=1) as wp,
            tc.tile_pool(name="qkv", bufs=1) as qkvp,
            tc.tile_pool(name="vaugp", bufs=1) as vap,
            tc.tile_pool(name="woutp", bufs=1) as wop,
            tc.tile_pool(name="dram", bufs=1, space="DRAM") as dp,
        ):
            ident_sb = cp.tile([P, P], F32, tag="ident")
            nc.sync.dma_start(ident_sb[:], ident_c[:, :])
            tri_sb = cp.tile([P, P], F32R, tag="tri")
            nc.sync.dma_start(tri_sb[:], tri_c[:, :].bitcast(F32R))
            selA_sb = cp.tile([1, P], F32R, tag="selA")
            nc.sync.dma_start(selA_sb[:], selA_c[:, :].bitcast(F32R))
            selB_sb = cp.tile([1, P], F32R, tag="selB")
            nc.sync.dma_start(selB_sb[:], selB_c[:, :].bitcast(F32R))
            ones_sb = cp.tile([P, 1], F32R, tag="ones")
            nc.sync.dma_start(ones_sb[:], ones_c[:, :].bitcast(F32R))
            b_sb = cp.tile([P, MW], F32, tag="bsh")
            nc.sync.dma_start(b_sb[:], b_sh[:, :])
            bob_sb = cp.tile([P, D], F32, tag="bob")
            nc.sync.dma_start(bob_sb[:], b_ob[:, :])

            w_sb = wp.tile([P, KT * MW * P], F32R, tag="wsh")
            for k in range(KT):
                nc.sync.dma_start(
                    w_sb[:, k * MW * P:(k + 1) * MW * P],
                    w_sh[k * P:(k + 1) * P, :].bitcast(F32R),
                )

            qT = qkvp.tile([P, T], F32R, tag="qT")
            kT = qkvp.tile([P, T], F32R, tag="kT")
            vT = qkvp.tile([P, T], F32, tag="vT")
            vaug = vap.tile([P, 2 * NBLK * VA], F32R, tag="vaug")

            a2a_in = dp.tile([NCORES, P, W], F32, tag="a2a_in")
            a2a_out = dp.tile([NCORES, P, W], F32, tag="a2a_out")

            # ---- Phase 1: qkvT projection (+ Phase 2: V transpose) ----
            with (
                tc.tile_pool(name="xcolp", bufs=2) as xp,
                tc.tile_pool(name="qkvps", bufs=2, space="PSUM") as qps,
                tc.tile_pool(name="vtps", bufs=2, space="PSUM") as vtps,
            ):
                for jj in range(T // W):
                    xcol = xp.tile([P, KT * W], F32R, tag="xcol")
                    for k in range(KT):
                        nc.sync.dma_start(
                            xcol[:, k * W:(k + 1) * W],
                            xT[k * P:(k + 1) * P, jj * W:(jj + 1) * W].bitcast(F32R),
                        )
                    for m in range(MW):
                        ps = qps.tile([P, W], F32, tag=f"qkv{m}")
                        for k in range(KT):
                            nc.tensor.matmul(
                                ps[:],
                                lhsT=w_sb[:, (k * MW + m) * P:(k * MW + m + 1) * P].bitcast(F32R),
                                rhs=xcol[:, k * W:(k + 1) * W].bitcast(F32R),
                                start=(k == 0),
                                stop=(k == KT - 1),
                            )
                        dest = (qT, kT, vT)[m]
                        nc.vector.tensor_scalar_add(
                            dest[:, jj * W:(jj + 1) * W], ps[:], b_sb[:, m:m + 1]
                        )
                    # V transpose for s-blocks covered by this column block
                    for sb in range(jj * W // SB, (jj + 1) * W // SB):
                        vps = vtps.tile([P, P], F32, tag="vt")
                        nc.tensor.transpose(
                            vps[:], vT[:, sb * SB:(sb + 1) * SB], ident_sb[:]
                        )
                        o = sb * VA
                        nc.vector.tensor_copy(vaug[:, o:o + DH], vps[:, 0:DH])
                        nc.vector.tensor_copy(
                            vaug[:, o + DH + 1:o + 2 * DH + 1], vps[:, DH:2 * DH]
                        )
                        nc.vector.tensor_copy(vaug[:, o + DH:o + DH + 1], ones_sb[:])
                        nc.vector.tensor_copy(vaug[:, o + 2 * DH + 1:o + VA], ones_sb[:])

            # ---- Phase 3: attention ----
            with (
                tc.tile_pool(name="stps", bufs=2, space="PSUM") as stps,
                tc.tile_pool(name="otps", bufs=1, space="PSUM") as otps,
                tc.tile_pool(name="bzps", bufs=1, space="PSUM") as bzps,
                tc.tile_pool(name="ptp", bufs=3) as ptp,
                tc.tile_pool(name="zp", bufs=2) as zp,
                tc.tile_pool(name="otnp", bufs=2) as otnp,
            ):
                for b in range(2):
                    for j in range(NW):
                        otA = otps.tile([DH + 1, W], F32, tag="otA")
                        otB = otps.tile([DH + 1, W], F32, tag="otB")
                        nblk = (j + 1) * W // SB
                        for i in range(nblk):
                            c0 = max(0, i * SB - j * W)
                            ks = slice(b * TB + i * SB, b * TB + (i + 1) * SB)
                            qs = slice(b * TB + j * W + c0, b * TB + (j + 1) * W)
                            stA = stps.tile([P, W], F32, tag="stA")
                            stB = stps.tile([P, W], F32, tag="stB")
                            nc.tensor.matmul(
                                stA[:, c0:], lhsT=kT[0:DH, ks].bitcast(F32R),
                                rhs=qT[0:DH, qs].bitcast(F32R),
                                start=True, stop=True, tile_position=(0, 0),
                            )
                            nc.tensor.matmul(
                                stB[:, c0:], lhsT=kT[DH:P, ks].bitcast(F32R),
                                rhs=qT[DH:P, qs].bitcast(F32R),
                                start=True, stop=True, tile_position=(64, 0),
                            )
                            ptA = ptp.tile([P, W], F32R, tag="ptA")
                            ptB = ptp.tile([P, W], F32R, tag="ptB")
                            nc.scalar.activation(ptA[:, c0:], stA[:, c0:], EXP,
                                                 scale=SCALE)
                            nc.scalar.activation(ptB[:, c0:], stB[:, c0:], EXP,
                                                 scale=SCALE)
                            if i * SB >= j * W:  # diagonal block: triangle mask
                                nc.vector.tensor_mul(
                                    ptA[:, c0:c0 + SB], ptA[:, c0:c0 + SB], tri_sb[:]
                                )
                                nc.vector.tensor_mul(
                                    ptB[:, c0:c0 + SB], ptB[:, c0:c0 + SB], tri_sb[:]
                                )
                            vo = (b * NBLK + i) * VA
                            nc.tensor.matmul(
                                otA[:, c0:], lhsT=vaug[:, vo:vo + DH + 1].bitcast(F32R),
                                rhs=ptA[:, c0:].bitcast(F32R),
                                start=(i == 0), stop=(i == nblk - 1),
                            )
                            nc.tensor.matmul(
                                otB[:, c0:],
                                lhsT=vaug[:, vo + DH + 1:vo + VA].bitcast(F32R),
                                rhs=ptB[:, c0:].bitcast(F32R),
                                start=(i == 0), stop=(i == nblk - 1),
                            )
                        # normalize by 1/Z (Z sits in PSUM row 64) + evacuate
                        zA = zp.tile([1, W], F32R, tag="zA")
                        zB = zp.tile([1, W], F32R, tag="zB")
                        nc.vector.tensor_copy(zA[:], otA[DH:DH + 1, :])
                        nc.vector.tensor_copy(zB[:], otB[DH:DH + 1, :])
                        bz = bzps.tile([P, W], F32, tag="bz")
                        nc.tensor.matmul(
                            bz[:], lhsT=selA_sb[:].bitcast(F32R),
                            rhs=zA[:].bitcast(F32R), start=True, stop=False,
                        )
                        nc.tensor.matmul(
                            bz[:], lhsT=selB_sb[:].bitcast(F32R),
                            rhs=zB[:].bitcast(F32R), start=False, stop=True,
                        )
                        rz = zp.tile([P, W], F32, tag="rz")
                        nc.vector.reciprocal(rz[:], bz[:])
                        otn = otnp.tile([P, W], F32, tag="otn")
                        nc.vector.tensor_mul(otn[0:DH, :], otA[0:DH, :], rz[0:DH, :])
                        nc.vector.tensor_mul(otn[DH:P, :], otB[0:DH, :], rz[DH:P, :])
                        shard = b * NW + j
                        nc.sync.dma_start(a2a_in[shard], otn[:])

            # ---- Phase 4: AllToAll (head shards -> token shards) ----
            nc.gpsimd.collective_compute(
                "AllToAll",
                mybir.AluOpType.bypass,
                replica_groups=[list(range(NCORES))],
                ins=[a2a_in[:]],
                outs=[a2a_out[:]],
            )

            # ---- Phase 5: out-projection for this core's W tokens ----
            wout_sb = wop.tile([P, KT * D], F32R, tag="wout")
            for k in range(KT):
                nc.sync.dma_start(
                    wout_sb[:, k * D:(k + 1) * D], w_out[k * P:(k + 1) * P, :].bitcast(F32R)
                )
            with (
                tc.tile_pool(name="lhsp", bufs=2 * KT) as lp,
                tc.tile_pool(name="yps", bufs=2, space="PSUM") as yps,
                tc.tile_pool(name="yp", bufs=2) as ypool,
            ):
                for tb in range(YT):
                    lhs = []
                    for r in range(NCORES):
                        lt = lp.tile([P, P], F32R, tag="lhs")
                        nc.sync.dma_start(
                            lt[:], a2a_out[r, :, tb * P:(tb + 1) * P].bitcast(F32R)
                        )
                        lhs.append(lt)
                    yt = ypool.tile([P, D], F32, tag="yt")
                    for mw in range(NMW):
                        ps = yps.tile([P, 512], F32, tag="yps")
                        for r in range(NCORES):
                            nc.tensor.matmul(
                                ps[:], lhsT=lhs[r][:].bitcast(F32R),
                                rhs=wout_sb[:, r * D + mw * 512:r * D + mw * 512 + 512].bitcast(F32R),
                                start=(r == 0), stop=(r == NCORES - 1),
                            )
                        nc.vector.tensor_add(
                            yt[:, mw * 512:(mw + 1) * 512], ps[:],
                            bob_sb[:, mw * 512:(mw + 1) * 512],
                        )
                    nc.sync.dma_start(y[tb * P:(tb + 1) * P, :], yt[:])

    nc.compile()
    return nc


def make_in_maps(x, w_qkv, b_qkv, w_out, b_out):
    """Host-side sharding/layout prep. x: [2, TB, D] float32."""
    x = np.asarray(x, np.float32)
    w_qkv = np.asarray(w_qkv, np.float32)
    b_qkv = np.asarray(b_qkv, np.float32)
    w_out = np.ascontiguousarray(np.asarray(w_out, np.float32))
    b_out = np.asarray(b_out, np.float32)
    nb, TB, d = x.shape
    assert nb == 2 and d == D
    T = nb * TB
    xT = np.ascontiguousarray(x.reshape(T, D).T)
    bob = np.ascontiguousarray(np.broadcast_to(b_out, (P, D)))
    in_maps = []
    for c in range(NCORES):
        h0 = c * P  # column offset of this core's 2 heads inside each block
        wsh = np.ascontiguousarray(np.concatenate(
            [w_qkv[:, h0:h0 + P],
             w_qkv[:, D + h0:D + h0 + P],
             w_qkv[:, 2 * D + h0:2 * D + h0 + P]], axis=1))
        bsh = np.ascontiguousarray(np.stack(
            [b_qkv[h0:h0 + P],
             b_qkv[D + h0:D + h0 + P],
             b_qkv[2 * D + h0:2 * D + h0 + P]], axis=1))
        in_maps.append({
            "xT": xT, "w_sh": wsh, "b_sh": bsh, "w_out": w_out, "b_ob": bob,
        })
    return in_maps, TB


_CACHE = {}


def _get_nc(TB, W):
    key = (TB, W)
    if key not in _CACHE:
        _CACHE[key] = build_nc(TB=TB, W=W)
    return _CACHE[key]


def _install_ntff_hook():
    """This image's antenv lacks axon_hooks; synthesize it so trace=True can
    drive NTFF profiling via the boot shim's ctypes path."""
    try:
        from antenv import axon_hooks  # noqa: F401
        return
    except ImportError:
        pass
    import types

    import antenv

    mod = types.ModuleType("antenv.axon_hooks")
    mod._hook = None
    mod.set_axon_ntff_profile_hook = lambda h: setattr(mod, "_hook", h)
    mod.get_axon_ntff_profile_hook = lambda: mod._hook
    sys.modules["antenv.axon_hooks"] = mod
    antenv.axon_hooks = mod
    try:
        from trn_agent_boot.trn_boot import _ntff_profile_via_ctypes

        hook = _ntff_profile_via_ctypes("/opt/axon/libaxon_pjrt.so")
        if hook is not None:
            mod._hook = hook
    except Exception as e:  # pragma: no cover
        print(f"ntff hook install failed: {e}", file=sys.stderr)


def run_device(x, w_qkv, b_qkv, w_out, b_out, trace=False):
    """Compile+run on the 8 NeuronCores; returns (y_full, BassKernelResults)."""
    import concourse.bass_utils as bu
    from concourse.bass_utils import run_bass_kernel_spmd

    if trace:
        _install_ntff_hook()
        bu.upload_artifacts = lambda tmpdir: f"local:{tmpdir}"

    in_maps, TB = make_in_maps(x, w_qkv, b_qkv, w_out, b_out)
    W = 2 * TB // NCORES
    nc = _get_nc(TB, W)
    res = run_bass_kernel_spmd(nc, in_maps, list(range(NCORES)), trace=trace)
    ys = [np.asarray(res.results[i]["y"]) for i in range(NCORES)]
    y = np.concatenate(ys, axis=0).reshape(2, TB, D).astype(np.float32)
    return y, res


def kernel(x, w_qkv, b_qkv, w_out, b_out):
    y, _ = run_device(x, w_qkv, b_qkv, w_out, b_out, trace=False)
    return y


# revision 14
# speedup vs baseline: 1.1879x; 1.1879x over previous
"""Multi-head causal self-attention Trainium2 kernel (8 NeuronCores, SPMD).

Strategy (head-parallel attention + token-parallel out-projection):
  - Host pre-transposes x -> xT [D, T] (T = 2*TB flattened tokens), casts to
    bf16, and slices per-core qkv weight shards (2 heads/core, columns
    [q2|k2|v2] -> [D, 384]).
  - Each core computes qkvT = w_shard^T @ x in feature-partitioned layout,
    giving qT/kT/vT [128, T] directly (2 heads stacked, 64+64).
  - V is re-transposed on the PE (T/128 small transposes) into natural [s, e]
    layout with an appended ones-column, so the PV matmul (M=65) accumulates
    the softmax denominator Z in PSUM row 64 for free.
  - Scores ST[s,t] = kT^T @ qT are computed transposed, 2 heads row-packed on
    the PE (K=64 each at array rows 0-63 / 64-127).  Causality is handled by
    shrinking the moving window (fully-masked columns are never computed) plus
    one constant 128x128 triangular mask multiply on diagonal blocks.
  - exp on ACT with the 1/8 scale folded into the activation scale operand; no
    running max needed (scores ~ N(0,1), so max over 2048 is ~4-5).
  - OT[e,t] is normalized by 1/Z via a K=2 broadcast matmul + DVE multiply,
    AllToAll redistributes head-shards -> token-shards (bf16, 1.05MB/rank),
    then each core out-projects its W-token slice against the full w_out.
  - Host concatenates the 8 per-core [W, D] outputs.

Matmul operands are bf16 (the PE moving-operand stream runs 2B/cycle, so
fp32/fp32r matmuls run at half rate); accumulation stays fp32 in PSUM.
"""

import os
import sys

import numpy as np

for _p in ("/opt/trn_rl_repo", "/root/.axon_site/_ro/trn_rl_repo"):
    if os.path.isdir(_p) and _p not in sys.path:
        sys.path.insert(0, _p)

import ml_dtypes  # noqa: E402

import concourse.bass as bass  # noqa: E402,F401
import concourse.mybir as mybir  # noqa: E402
import concourse.tile as tile  # noqa: E402
from concourse import bacc  # noqa: E402

F32 = mybir.dt.float32
BF16 = mybir.dt.bfloat16
EXP = mybir.ActivationFunctionType.Exp
NPBF = ml_dtypes.bfloat16

D = 1024         # model dim
DH = 64          # head dim
NCORES = 8
P = 128          # partitions
MW = 3           # m-tiles of the qkv shard: q(2 heads)|k|v, each 128 wide
SB = 128         # s-block (key block)
SCALE = DH ** -0.5

VA = 2 * DH + 2  # V_aug stride: [V_A(64) | ones | V_B(64) | ones] = 130


def build_nc(TB=2048, W=512):
    """Build the SPMD Bass program. TB = tokens per batch, W = token window."""
    T = 2 * TB                  # flattened tokens (2 batches)
    NW = TB // W                # windows per batch
    assert T % W == 0 and T // W == NCORES, "A2A shards must equal cores"
    KT = D // P                 # contraction tiles (8)
    NBLK = TB // SB             # s-blocks per batch
    YT = W // P                 # out-proj token tiles per core
    NMW = D // 512              # out-proj N windows

    nc = bacc.Bacc(
        "TRN2", target_bir_lowering=False, debug=False, num_devices=NCORES
    )

    xT = nc.dram_tensor("xT", [D, T], BF16, kind="ExternalInput")
    w_sh = nc.dram_tensor("w_sh", [D, MW * P], BF16, kind="ExternalInput")
    b_sh = nc.dram_tensor("b_sh", [P, MW], F32, kind="ExternalInput")
    w_out = nc.dram_tensor("w_out", [D, D], BF16, kind="ExternalInput")
    b_ob = nc.dram_tensor("b_ob", [P, D], F32, kind="ExternalInput")
    y = nc.dram_tensor("y", [W, D], F32, kind="ExternalOutput")

    ident_c = nc.inline_tensor(np.eye(P, dtype=NPBF), "ident_c")
    tri_np = (np.arange(P)[:, None] <= np.arange(P)[None, :]).astype(NPBF)
    tri_c = nc.inline_tensor(tri_np, "tri_c")
    selA_np = np.zeros((1, P), NPBF)
    selA_np[0, :DH] = 1.0
    selA_c = nc.inline_tensor(selA_np, "selA_c")
    selB_np = np.zeros((1, P), NPBF)
    selB_np[0, DH:] = 1.0
    selB_c = nc.inline_tensor(selB_np, "selB_c")
    ones_c = nc.inline_tensor(np.ones((P, 1), NPBF), "ones_c")

    with tile.TileContext(nc) as tc:
        with (
            tc.tile_pool(name="consts", bufs=1) as cp,
            tc.tile_pool(name="wq", bufs=1) as wp,
            tc.tile_pool(name="qkv", bufs=1) as qkvp,
            tc.tile_pool(name="vaugp", bufs=1) as vap,
            tc.tile_pool(name="woutp", bufs=1) as wop,
            tc.tile_pool(name="dram", bufs=1, space="DRAM") as dp,
        ):
            ident_sb = cp.tile([P, P], BF16, tag="ident")
            nc.sync.dma_start(ident_sb[:], ident_c[:, :])
            tri_sb = cp.tile([P, P], BF16, tag="tri")
            nc.sync.dma_start(tri_sb[:], tri_c[:, :])
            selA_sb = cp.tile([1, P], BF16, tag="selA")
            nc.sync.dma_start(selA_sb[:], selA_c[:, :])
            selB_sb = cp.tile([1, P], BF16, tag="selB")
            nc.sync.dma_start(selB_sb[:], selB_c[:, :])
            ones_sb = cp.tile([P, 1], BF16, tag="ones")
            nc.sync.dma_start(ones_sb[:], ones_c[:, :])
            b_sb = cp.tile([P, MW], F32, tag="bsh")
            nc.sync.dma_start(b_sb[:], b_sh[:, :])
            bob_sb = cp.tile([P, D], F32, tag="bob")
            nc.sync.dma_start(bob_sb[:], b_ob[:, :])

            w_sb = wp.tile([P, KT * MW * P], BF16, tag="wsh")
            for k in range(KT):
                nc.sync.dma_start(
                    w_sb[:, k * MW * P:(k + 1) * MW * P],
                    w_sh[k * P:(k + 1) * P, :],
                )

            qT = qkvp.tile([P, T], BF16, tag="qT")
            kT = qkvp.tile([P, T], BF16, tag="kT")
            vT = qkvp.tile([P, T], BF16, tag="vT")
            vaug = vap.tile([P, 2 * NBLK * VA], BF16, tag="vaug")

            a2a_in = dp.tile([NCORES, P, W], BF16, tag="a2a_in")
            a2a_out = dp.tile([NCORES, P, W], BF16, tag="a2a_out")

            # ---- Phase 1: qkvT projection (+ Phase 2: V transpose) ----
            with (
                tc.tile_pool(name="xcolp", bufs=3) as xp,
                tc.tile_pool(name="qkvps", bufs=2, space="PSUM") as qps,
                tc.tile_pool(name="vtps", bufs=2, space="PSUM") as vtps,
            ):
                for jj in range(T // W):
                    xcol = xp.tile([P, KT * W], BF16, tag="xcol")
                    for k in range(KT):
                        nc.sync.dma_start(
                            xcol[:, k * W:(k + 1) * W],
                            xT[k * P:(k + 1) * P, jj * W:(jj + 1) * W],
                        )
                    for m in range(MW):
                        ps = qps.tile([P, W], F32, tag=f"qkv{m}")
                        for k in range(KT):
                            nc.tensor.matmul(
                                ps[:],
                                lhsT=w_sb[:, (k * MW + m) * P:(k * MW + m + 1) * P],
                                rhs=xcol[:, k * W:(k + 1) * W],
                                start=(k == 0),
                                stop=(k == KT - 1),
                            )
                        dest = (qT, kT, vT)[m]
                        nc.vector.tensor_scalar_add(
                            dest[:, jj * W:(jj + 1) * W], ps[:], b_sb[:, m:m + 1]
                        )
                    # V transpose for s-blocks covered by this column block
                    for sb in range(jj * W // SB, (jj + 1) * W // SB):
                        vps = vtps.tile([P, P], BF16, tag="vt")
                        nc.tensor.transpose(
                            vps[:], vT[:, sb * SB:(sb + 1) * SB], ident_sb[:]
                        )
                        o = sb * VA
                        nc.vector.tensor_copy(vaug[:, o:o + DH], vps[:, 0:DH])
                        nc.vector.tensor_copy(
                            vaug[:, o + DH + 1:o + 2 * DH + 1], vps[:, DH:2 * DH]
                        )
                        nc.vector.tensor_copy(vaug[:, o + DH:o + DH + 1], ones_sb[:])
                        nc.vector.tensor_copy(
                            vaug[:, o + 2 * DH + 1:o + VA], ones_sb[:]
                        )

            # ---- Phase 3: attention ----
            with (
                tc.tile_pool(name="stps", bufs=2, space="PSUM") as stps,
                tc.tile_pool(name="otps", bufs=2, space="PSUM") as otps,
                tc.tile_pool(name="ptp", bufs=3) as ptp,
                tc.tile_pool(name="zp", bufs=2) as zp,
                tc.tile_pool(name="otnp", bufs=2) as otnp,
            ):
                for b in range(2):
                    for j in range(NW):
                        otA = otps.tile([DH + 1, W], F32, tag="otA")
                        otB = otps.tile([DH + 1, W], F32, tag="otB")
                        nblk = (j + 1) * W // SB
                        for i in range(nblk):
                            c0 = max(0, i * SB - j * W)
                            ks = slice(b * TB + i * SB, b * TB + (i + 1) * SB)
                            qs = slice(b * TB + j * W + c0, b * TB + (j + 1) * W)
                            # both heads in one [128, 2W] tile: A=[:, :W], B=[:, W:]
                            st = stps.tile([P, 2 * W], F32, tag="st")
                            nc.tensor.matmul(
                                st[:, c0:W], lhsT=kT[0:DH, ks],
                                rhs=qT[0:DH, qs],
                                start=True, stop=True, tile_position=(0, 0),
                            )
                            nc.tensor.matmul(
                                st[:, W + c0:], lhsT=kT[DH:P, ks],
                                rhs=qT[DH:P, qs],
                                start=True, stop=True, tile_position=(64, 0),
                            )
                            pt = ptp.tile([P, 2 * W], BF16, tag="pt")
                            # one ACT op covers both heads' valid sub-windows
                            st3 = st[:, :].rearrange("p (h w) -> p h w", h=2)
                            pt3 = pt[:, :].rearrange("p (h w) -> p h w", h=2)
                            nc.scalar.activation(
                                pt3[:, :, c0:], st3[:, :, c0:], EXP, scale=SCALE
                            )
                            if i * SB >= j * W:  # diagonal block: triangle mask
                                nc.vector.tensor_mul(
                                    pt[:, c0:c0 + SB], pt[:, c0:c0 + SB], tri_sb[:]
                                )
                                nc.vector.tensor_mul(
                                    pt[:, W + c0:W + c0 + SB],
                                    pt[:, W + c0:W + c0 + SB], tri_sb[:],
                                )
                            vo = (b * NBLK + i) * VA
                            nc.tensor.matmul(
                                otA[:, c0:], lhsT=vaug[:, vo:vo + DH + 1],
                                rhs=pt[:, c0:W],
                                start=(i == 0), stop=(i == nblk - 1),
                            )
                            nc.tensor.matmul(
                                otB[:, c0:],
                                lhsT=vaug[:, vo + DH + 1:vo + VA],
                                rhs=pt[:, W + c0:],
                                start=(i == 0), stop=(i == nblk - 1),
                            )
                        # normalize by 1/Z (Z sits in PSUM row 64) + evacuate
                        zA = zp.tile([1, W], BF16, tag="zA")
                        zB = zp.tile([1, W], BF16, tag="zB")
                        nc.vector.tensor_copy(zA[:], otA[DH:DH + 1, :])
                        nc.vector.tensor_copy(zB[:], otB[DH:DH + 1, :])
                        # reuse an st slot (keeps PSUM within 8 banks)
                        bzt = stps.tile([P, 2 * W], F32, tag="st", name="bzt")
                        bz = bzt[:, 0:W]
                        nc.tensor.matmul(
                            bz[:], lhsT=selA_sb[:], rhs=zA[:],
                            start=True, stop=False,
                        )
                        nc.tensor.matmul(
                            bz[:], lhsT=selB_sb[:], rhs=zB[:],
                            start=False, stop=True,
                        )
                        rz = zp.tile([P, W], F32, tag="rz")
                        nc.vector.reciprocal(rz[:], bz[:])
                        otn = otnp.tile([P, W], BF16, tag="otn")
                        nc.vector.tensor_mul(otn[0:DH, :], otA[0:DH, :], rz[0:DH, :])
                        nc.vector.tensor_mul(otn[DH:P, :], otB[0:DH, :], rz[DH:P, :])
                        shard = b * NW + j
                        nc.sync.dma_start(a2a_in[shard], otn[:])

            # ---- Phase 4: AllToAll (head shards -> token shards) ----
            nc.gpsimd.collective_compute(
                "AllToAll",
                mybir.AluOpType.bypass,
                replica_groups=[list(range(NCORES))],
                ins=[a2a_in[:]],
                outs=[a2a_out[:]],
            )

            # ---- Phase 5: out-projection for this core's W tokens ----
            wout_sb = wop.tile([P, KT * D], BF16, tag="wout")
            for k in range(KT):
                nc.sync.dma_start(
                    wout_sb[:, k * D:(k + 1) * D], w_out[k * P:(k + 1) * P, :]
                )
            with (
                tc.tile_pool(name="lhsp", bufs=2 * KT) as lp,
                tc.tile_pool(name="yps", bufs=2, space="PSUM") as yps,
                tc.tile_pool(name="yp", bufs=2) as ypool,
            ):
                for tb in range(YT):
                    lhs = []
                    for r in range(NCORES):
                        lt = lp.tile([P, P], BF16, tag="lhs")
                        nc.sync.dma_start(
                            lt[:], a2a_out[r, :, tb * P:(tb + 1) * P]
                        )
                        lhs.append(lt)
                    yt = ypool.tile([P, D], F32, tag="yt")
                    for mw in range(NMW):
                        ps = yps.tile([P, 512], F32, tag="yps")
                        for r in range(NCORES):
                            nc.tensor.matmul(
                                ps[:], lhsT=lhs[r][:],
                                rhs=wout_sb[:, r * D + mw * 512:r * D + mw * 512 + 512],
                                start=(r == 0), stop=(r == NCORES - 1),
                            )
                        nc.vector.tensor_add(
                            yt[:, mw * 512:(mw + 1) * 512], ps[:],
                            bob_sb[:, mw * 512:(mw + 1) * 512],
                        )
                    nc.sync.dma_start(y[tb * P:(tb + 1) * P, :], yt[:])

    nc.compile()
    return nc


def make_in_maps(x, w_qkv, b_qkv, w_out, b_out):
    """Host-side sharding/layout prep. x: [2, TB, D] float32."""
    x = np.asarray(x, np.float32)
    w_qkv = np.asarray(w_qkv, np.float32)
    b_qkv = np.asarray(b_qkv, np.float32)
    w_out = np.ascontiguousarray(np.asarray(w_out, np.float32).astype(NPBF))
    b_out = np.asarray(b_out, np.float32)
    nb, TB, d = x.shape
    assert nb == 2 and d == D
    T = nb * TB
    xT = np.ascontiguousarray(x.reshape(T, D).T.astype(NPBF))
    bob = np.ascontiguousarray(np.broadcast_to(b_out, (P, D)).astype(np.float32))
    in_maps = []
    for c in range(NCORES):
        h0 = c * P  # column offset of this core's 2 heads inside each block
        wsh = np.ascontiguousarray(np.concatenate(
            [w_qkv[:, h0:h0 + P],
             w_qkv[:, D + h0:D + h0 + P],
             w_qkv[:, 2 * D + h0:2 * D + h0 + P]], axis=1).astype(NPBF))
        bsh = np.ascontiguousarray(np.stack(
            [b_qkv[h0:h0 + P],
             b_qkv[D + h0:D + h0 + P],
             b_qkv[2 * D + h0:2 * D + h0 + P]], axis=1))
        in_maps.append({
            "xT": xT, "w_sh": wsh, "b_sh": bsh, "w_out": w_out, "b_ob": bob,
        })
    return in_maps, TB


_CACHE = {}


def _get_nc(TB, W):
    key = (TB, W)
    if key not in _CACHE:
        _CACHE[key] = build_nc(TB=TB, W=W)
    return _CACHE[key]


def _install_ntff_hook():
    """This image's antenv lacks axon_hooks; synthesize it so trace=True can
    drive NTFF profiling via the boot shim's ctypes path."""
    try:
        from antenv import axon_hooks  # noqa: F401
        return
    except ImportError:
        pass
    import types

    import antenv

    mod = types.ModuleType("antenv.axon_hooks")
    mod._hook = None
    mod.set_axon_ntff_profile_hook = lambda h: setattr(mod, "_hook", h)
    mod.get_axon_ntff_profile_hook = lambda: mod._hook
    sys.modules["antenv.axon_hooks"] = mod
    antenv.axon_hooks = mod
    try:
        from trn_agent_boot.trn_boot import _ntff_profile_via_ctypes

        hook = _ntff_profile_via_ctypes("/opt/axon/libaxon_pjrt.so")
        if hook is not None:
            mod._hook = hook
    except Exception as e:  # pragma: no cover
        print(f"ntff hook install failed: {e}", file=sys.stderr)


def run_device(x, w_qkv, b_qkv, w_out, b_out, trace=False):
    """Compile+run on the 8 NeuronCores; returns (y_full, BassKernelResults)."""
    import concourse.bass_utils as bu
    from concourse.bass_utils import run_bass_kernel_spmd

    if trace:
        _install_ntff_hook()
        bu.upload_artifacts = lambda tmpdir: f"local:{tmpdir}"

    in_maps, TB = make_in_maps(x, w_qkv, b_qkv, w_out, b_out)
    W = 2 * TB // NCORES
    nc = _get_nc(TB, W)
    res = run_bass_kernel_spmd(nc, in_maps, list(range(NCORES)), trace=trace)
    ys = [np.asarray(res.results[i]["y"]) for i in range(NCORES)]
    y = np.concatenate(ys, axis=0).reshape(2, TB, D).astype(np.float32)
    return y, res


def kernel(x, w_qkv, b_qkv, w_out, b_out):
    y, _ = run_device(x, w_qkv, b_qkv, w_out, b_out, trace=False)
    return y


# revision 17
# speedup vs baseline: 1.4195x; 1.1950x over previous
"""Multi-head causal self-attention Trainium2 kernel (8 NeuronCores, SPMD).

Strategy (head-parallel attention + token-parallel out-projection):
  - Host pre-transposes x -> xT [D, T] (T = 2*TB flattened tokens), casts to
    bf16, and slices per-core qkv weight shards (2 heads/core, columns
    [q2|k2|v2] -> [D, 384]).
  - Each core computes qkvT = w_shard^T @ x in feature-partitioned layout,
    giving qT/kT/vT [128, T] directly (2 heads stacked, 64+64).
  - V is re-transposed on the PE (T/128 small transposes) into natural [s, e]
    layout with an appended ones-column, so the PV matmul (M=65) accumulates
    the softmax denominator Z in PSUM row 64 for free.
  - Scores ST[s,t] = kT^T @ qT are computed transposed, 2 heads row-packed on
    the PE (K=64 each at array rows 0-63 / 64-127).  Causality is handled by
    shrinking the moving window (fully-masked columns are never computed) plus
    one constant 128x128 triangular mask multiply on diagonal blocks.
  - exp on ACT with the 1/8 scale folded into the activation scale operand; no
    running max needed (scores ~ N(0,1), so max over 2048 is ~4-5).
  - OT[e,t] is normalized by 1/Z via a K=2 broadcast matmul + DVE multiply,
    AllToAll redistributes head-shards -> token-shards (bf16, 1.05MB/rank),
    then each core out-projects its W-token slice against the full w_out.
  - Host concatenates the 8 per-core [W, D] outputs.

Matmul operands are bf16 (the PE moving-operand stream runs 2B/cycle, so
fp32/fp32r matmuls run at half rate); accumulation stays fp32 in PSUM.
"""

import os
import sys

import numpy as np

for _p in ("/opt/trn_rl_repo", "/root/.axon_site/_ro/trn_rl_repo"):
    if os.path.isdir(_p) and _p not in sys.path:
        sys.path.insert(0, _p)

import ml_dtypes  # noqa: E402

import concourse.bass as bass  # noqa: E402,F401
import concourse.mybir as mybir  # noqa: E402
import concourse.tile as tile  # noqa: E402
from concourse import bacc  # noqa: E402

F32 = mybir.dt.float32
BF16 = mybir.dt.bfloat16
EXP = mybir.ActivationFunctionType.Exp
NPBF = ml_dtypes.bfloat16

D = 1024         # model dim
DH = 64          # head dim
NCORES = 8
P = 128          # partitions
MW = 3           # m-tiles of the qkv shard: q(2 heads)|k|v, each 128 wide
SB = 128         # s-block (key block)
SCALE = DH ** -0.5

VA = 2 * DH + 2  # V_aug stride: [V_A(64) | ones | V_B(64) | ones] = 130


def build_nc(TB=2048, W=512):
    """Build the SPMD Bass program. TB = tokens per batch, W = token window."""
    T = 2 * TB                  # flattened tokens (2 batches)
    NW = TB // W                # windows per batch
    assert T % W == 0 and T // W == NCORES, "A2A shards must equal cores"
    KT = D // P                 # contraction tiles (8)
    NBLK = TB // SB             # s-blocks per batch
    YT = W // P                 # out-proj token tiles per core
    NMW = D // 512              # out-proj N windows

    nc = bacc.Bacc(
        "TRN2", target_bir_lowering=False, debug=False, num_devices=NCORES
    )

    xT = nc.dram_tensor("xT", [D, T], BF16, kind="ExternalInput")
    w_sh = nc.dram_tensor("w_sh", [D, MW * P], BF16, kind="ExternalInput")
    b_sh = nc.dram_tensor("b_sh", [P, MW], F32, kind="ExternalInput")
    w_out = nc.dram_tensor("w_out", [D, D], BF16, kind="ExternalInput")
    b_ob = nc.dram_tensor("b_ob", [P, D], F32, kind="ExternalInput")
    y = nc.dram_tensor("y", [W, D], F32, kind="ExternalOutput")

    ident_c = nc.inline_tensor(np.eye(P, dtype=NPBF), "ident_c")
    tri_np = (np.arange(P)[:, None] <= np.arange(P)[None, :]).astype(NPBF)
    tri_c = nc.inline_tensor(tri_np, "tri_c")
    selA_np = np.zeros((1, P), NPBF)
    selA_np[0, :DH] = 1.0
    selA_c = nc.inline_tensor(selA_np, "selA_c")
    selB_np = np.zeros((1, P), NPBF)
    selB_np[0, DH:] = 1.0
    selB_c = nc.inline_tensor(selB_np, "selB_c")
    ones_c = nc.inline_tensor(np.ones((P, 1), NPBF), "ones_c")

    with tile.TileContext(nc) as tc:
        with (
            tc.tile_pool(name="consts", bufs=1) as cp,
            tc.tile_pool(name="wq", bufs=1) as wp,
            tc.tile_pool(name="qkv", bufs=1) as qkvp,
            tc.tile_pool(name="vaugp", bufs=1) as vap,
            tc.tile_pool(name="woutp", bufs=1) as wop,
            tc.tile_pool(name="dram", bufs=1, space="DRAM") as dp,
        ):
            w_sb = wp.tile([P, KT * MW * P], BF16, tag="wsh")
            nc.sync.dma_start(
                w_sb[:, :].rearrange("p (k c) -> p k c", k=KT),
                w_sh[:, :].rearrange("(k p) c -> p k c", p=P),
            )
            ident_sb = cp.tile([P, P], BF16, tag="ident")
            nc.sync.dma_start(ident_sb[:], ident_c[:, :])
            tri_sb = cp.tile([P, P], BF16, tag="tri")
            nc.sync.dma_start(tri_sb[:], tri_c[:, :])
            selA_sb = cp.tile([1, P], BF16, tag="selA")
            nc.sync.dma_start(selA_sb[:], selA_c[:, :])
            selB_sb = cp.tile([1, P], BF16, tag="selB")
            nc.sync.dma_start(selB_sb[:], selB_c[:, :])
            ones_sb = cp.tile([P, 1], BF16, tag="ones")
            nc.sync.dma_start(ones_sb[:], ones_c[:, :])
            b_sb = cp.tile([P, MW], F32, tag="bsh")
            nc.sync.dma_start(b_sb[:], b_sh[:, :])
            bob_sb = cp.tile([P, D], F32, tag="bob")
            nc.sync.dma_start(bob_sb[:], b_ob[:, :])

            qT = qkvp.tile([P, T], BF16, tag="qT")
            kT = qkvp.tile([P, T], BF16, tag="kT")
            vT = qkvp.tile([P, T], BF16, tag="vT")
            vaug = vap.tile([P, 2 * NBLK * VA], BF16, tag="vaug")

            a2a_in = dp.tile([NCORES, P, W], BF16, tag="a2a_in")
            a2a_out = dp.tile([NCORES, P, W], BF16, tag="a2a_out")

            # ---- Phase 1: qkvT projection (+ Phase 2: V transpose) ----
            with (
                tc.tile_pool(name="xcolp", bufs=3) as xp,
                tc.tile_pool(name="qkvps", bufs=2, space="PSUM") as qps,
                tc.tile_pool(name="vtps", bufs=2, space="PSUM") as vtps,
            ):
                for jj in range(T // W):
                    xcol = xp.tile([P, KT * W], BF16, tag="xcol")
                    nc.sync.dma_start(
                        xcol[:, :].rearrange("p (k w) -> p k w", k=KT),
                        xT[:, jj * W:(jj + 1) * W].rearrange(
                            "(k p) w -> p k w", p=P
                        ),
                    )
                    for m in range(MW):
                        ps = qps.tile([P, W], F32, tag=f"qkv{m}")
                        for k in range(KT):
                            nc.tensor.matmul(
                                ps[:],
                                lhsT=w_sb[:, (k * MW + m) * P:(k * MW + m + 1) * P],
                                rhs=xcol[:, k * W:(k + 1) * W],
                                start=(k == 0),
                                stop=(k == KT - 1),
                            )
                        dest = (qT, kT, vT)[m]
                        nc.scalar.add(
                            dest[:, jj * W:(jj + 1) * W], ps[:], b_sb[:, m:m + 1]
                        )
                    # V transpose for s-blocks covered by this column block
                    for sb in range(jj * W // SB, (jj + 1) * W // SB):
                        vps = vtps.tile([P, P], BF16, tag="vt")
                        nc.tensor.transpose(
                            vps[:], vT[:, sb * SB:(sb + 1) * SB], ident_sb[:]
                        )
                        o = sb * VA
                        nc.vector.tensor_copy(vaug[:, o:o + DH], vps[:, 0:DH])
                        nc.vector.tensor_copy(
                            vaug[:, o + DH + 1:o + 2 * DH + 1], vps[:, DH:2 * DH]
                        )
                        nc.vector.tensor_copy(vaug[:, o + DH:o + DH + 1], ones_sb[:])
                        nc.vector.tensor_copy(
                            vaug[:, o + 2 * DH + 1:o + VA], ones_sb[:]
                        )

            # ---- Phase 3: attention ----
            with (
                tc.tile_pool(name="stps", bufs=2, space="PSUM") as stps,
                tc.tile_pool(name="otps", bufs=2, space="PSUM") as otps,
                tc.tile_pool(name="ptp", bufs=3) as ptp,
                tc.tile_pool(name="zp", bufs=2) as zp,
                tc.tile_pool(name="otnp", bufs=2) as otnp,
            ):
                def emit_norm(state):
                    """Normalize window state's OT by 1/Z and ship to a2a_in.

                    Runs on DVE + GpSimd only (PE/PSUM stay free): copy the Z
                    rows out of PSUM, partition-broadcast them on GpSimd, one
                    fast reciprocal, two multiplies, one DMA."""
                    otA_, otB_, b_, j_ = state
                    zA = zp.tile([1, W], F32, tag="zA")
                    zB = zp.tile([1, W], F32, tag="zB")
                    nc.vector.tensor_copy(zA[:], otA_[DH:DH + 1, :])
                    nc.vector.tensor_copy(zB[:], otB_[DH:DH + 1, :])
                    zbcA = zp.tile([DH, W], F32, tag="zbcA")
                    zbcB = zp.tile([DH, W], F32, tag="zbcB")
                    nc.gpsimd.partition_broadcast(zbcA[:], zA[:])
                    nc.gpsimd.partition_broadcast(zbcB[:], zB[:])
                    rzA = zp.tile([DH, W], F32, tag="rzA")
                    rzB = zp.tile([DH, W], F32, tag="rzB")
                    nc.vector.reciprocal_approx_fast(rzA[:], zbcA[:])
                    nc.vector.reciprocal_approx_fast(rzB[:], zbcB[:])
                    otn = otnp.tile([P, W], BF16, tag="otn")
                    nc.vector.tensor_mul(otn[0:DH, :], otA_[0:DH, :], rzA[:])
                    nc.vector.tensor_mul(otn[DH:P, :], otB_[0:DH, :], rzB[:])
                    nc.sync.dma_start(a2a_in[b_ * NW + j_], otn[:])

                pending = None
                for b in range(2):
                    for j in range(NW):
                        otA = otps.tile([DH + 1, W], F32, tag="otA")
                        otB = otps.tile([DH + 1, W], F32, tag="otB")
                        nblk = (j + 1) * W // SB
                        for i in range(nblk):
                            c0 = max(0, i * SB - j * W)
                            ks = slice(b * TB + i * SB, b * TB + (i + 1) * SB)
                            qs = slice(b * TB + j * W + c0, b * TB + (j + 1) * W)
                            # both heads in one [128, 2W] tile: A=[:, :W], B=[:, W:]
                            st = stps.tile([P, 2 * W], F32, tag="st")
                            nc.tensor.matmul(
                                st[:, c0:W], lhsT=kT[0:DH, ks],
                                rhs=qT[0:DH, qs],
                                start=True, stop=True, tile_position=(0, 0),
                            )
                            nc.tensor.matmul(
                                st[:, W + c0:], lhsT=kT[DH:P, ks],
                                rhs=qT[DH:P, qs],
                                start=True, stop=True, tile_position=(64, 0),
                            )
                            pt = ptp.tile([P, 2 * W], BF16, tag="pt")
                            # one ACT op covers both heads' valid sub-windows
                            st3 = st[:, :].rearrange("p (h w) -> p h w", h=2)
                            pt3 = pt[:, :].rearrange("p (h w) -> p h w", h=2)
                            nc.scalar.activation(
                                pt3[:, :, c0:], st3[:, :, c0:], EXP, scale=SCALE
                            )
                            if i * SB >= j * W:  # diagonal block: triangle mask
                                nc.vector.tensor_mul(
                                    pt[:, c0:c0 + SB], pt[:, c0:c0 + SB], tri_sb[:]
                                )
                                nc.vector.tensor_mul(
                                    pt[:, W + c0:W + c0 + SB],
                                    pt[:, W + c0:W + c0 + SB], tri_sb[:],
                                )
                            vo = (b * NBLK + i) * VA
                            nc.tensor.matmul(
                                otA[:, c0:], lhsT=vaug[:, vo:vo + DH + 1],
                                rhs=pt[:, c0:W],
                                start=(i == 0), stop=(i == nblk - 1),
                            )
                            nc.tensor.matmul(
                                otB[:, c0:],
                                lhsT=vaug[:, vo + DH + 1:vo + VA],
                                rhs=pt[:, W + c0:],
                                start=(i == 0), stop=(i == nblk - 1),
                            )
                            if pending is not None and (i == 2 or i == nblk - 1):
                                # software-pipelined: previous window's
                                # normalization trails into this window
                                emit_norm(pending)
                                pending = None
                        pending = (otA, otB, b, j)
                emit_norm(pending)

            # ---- Phase 4: AllToAll (head shards -> token shards) ----
            nc.gpsimd.collective_compute(
                "AllToAll",
                mybir.AluOpType.bypass,
                replica_groups=[list(range(NCORES))],
                ins=[a2a_in[:]],
                outs=[a2a_out[:]],
            )

            # ---- Phase 5: out-projection for this core's W tokens ----
            wout_sb = wop.tile([P, KT * D], BF16, tag="wout")
            nc.sync.dma_start(
                wout_sb[:, :].rearrange("p (k c) -> p k c", k=KT),
                w_out[:, :].rearrange("(k p) c -> p k c", p=P),
            )
            with (
                tc.tile_pool(name="lhsp", bufs=2) as lp,
                tc.tile_pool(name="yps", bufs=2, space="PSUM") as yps,
                tc.tile_pool(name="yp", bufs=2) as ypool,
            ):
                for tb in range(YT):
                    lhsb = lp.tile([P, NCORES * P], BF16, tag="lhs")
                    nc.sync.dma_start(
                        lhsb[:, :].rearrange("p (r c) -> p r c", r=NCORES),
                        a2a_out[:, :, tb * P:(tb + 1) * P].rearrange(
                            "r p c -> p r c"
                        ),
                    )
                    yt = ypool.tile([P, D], F32, tag="yt")
                    for mw in range(NMW):
                        ps = yps.tile([P, 512], F32, tag="yps")
                        for r in range(NCORES):
                            nc.tensor.matmul(
                                ps[:], lhsT=lhsb[:, r * P:(r + 1) * P],
                                rhs=wout_sb[:, r * D + mw * 512:r * D + mw * 512 + 512],
                                start=(r == 0), stop=(r == NCORES - 1),
                            )
                        nc.vector.tensor_add(
                            yt[:, mw * 512:(mw + 1) * 512], ps[:],
                            bob_sb[:, mw * 512:(mw + 1) * 512],
                        )
                    nc.sync.dma_start(y[tb * P:(tb + 1) * P, :], yt[:])

    nc.compile()
    return nc


def make_in_maps(x, w_qkv, b_qkv, w_out, b_out):
    """Host-side sharding/layout prep. x: [2, TB, D] float32."""
    x = np.asarray(x, np.float32)
    w_qkv = np.asarray(w_qkv, np.float32)
    b_qkv = np.asarray(b_qkv, np.float32)
    w_out = np.ascontiguousarray(np.asarray(w_out, np.float32).astype(NPBF))
    b_out = np.asarray(b_out, np.float32)
    nb, TB, d = x.shape
    assert nb == 2 and d == D
    T = nb * TB
    xT = np.ascontiguousarray(x.reshape(T, D).T.astype(NPBF))
    bob = np.ascontiguousarray(np.broadcast_to(b_out, (P, D)).astype(np.float32))
    in_maps = []
    for c in range(NCORES):
        h0 = c * P  # column offset of this core's 2 heads inside each block
        wsh = np.ascontiguousarray(np.concatenate(
            [w_qkv[:, h0:h0 + P],
             w_qkv[:, D + h0:D + h0 + P],
             w_qkv[:, 2 * D + h0:2 * D + h0 + P]], axis=1).astype(NPBF))
        bsh = np.ascontiguousarray(np.stack(
            [b_qkv[h0:h0 + P],
             b_qkv[D + h0:D + h0 + P],
             b_qkv[2 * D + h0:2 * D + h0 + P]], axis=1))
        in_maps.append({
            "xT": xT, "w_sh": wsh, "b_sh": bsh, "w_out": w_out, "b_ob": bob,
        })
    return in_maps, TB


_CACHE = {}


def _get_nc(TB, W):
    key = (TB, W)
    if key not in _CACHE:
        _CACHE[key] = build_nc(TB=TB, W=W)
    return _CACHE[key]


def _install_ntff_hook():
    """This image's antenv lacks axon_hooks; synthesize it so trace=True can
    drive NTFF profiling via the boot shim's ctypes path."""
    try:
        from antenv import axon_hooks  # noqa: F401
        return
    except ImportError:
        pass
    import types

    import antenv

    mod = types.ModuleType("antenv.axon_hooks")
    mod._hook = None
    mod.set_axon_ntff_profile_hook = lambda h: setattr(mod, "_hook", h)
    mod.get_axon_ntff_profile_hook = lambda: mod._hook
    sys.modules["antenv.axon_hooks"] = mod
    antenv.axon_hooks = mod
    try:
        from trn_agent_boot.trn_boot import _ntff_profile_via_ctypes

        hook = _ntff_profile_via_ctypes("/opt/axon/libaxon_pjrt.so")
        if hook is not None:
            mod._hook = hook
    except Exception as e:  # pragma: no cover
        print(f"ntff hook install failed: {e}", file=sys.stderr)


def run_device(x, w_qkv, b_qkv, w_out, b_out, trace=False):
    """Compile+run on the 8 NeuronCores; returns (y_full, BassKernelResults)."""
    import concourse.bass_utils as bu
    from concourse.bass_utils import run_bass_kernel_spmd

    if trace:
        _install_ntff_hook()
        bu.upload_artifacts = lambda tmpdir: f"local:{tmpdir}"

    in_maps, TB = make_in_maps(x, w_qkv, b_qkv, w_out, b_out)
    W = 2 * TB // NCORES
    nc = _get_nc(TB, W)
    res = run_bass_kernel_spmd(nc, in_maps, list(range(NCORES)), trace=trace)
    ys = [np.asarray(res.results[i]["y"]) for i in range(NCORES)]
    y = np.concatenate(ys, axis=0).reshape(2, TB, D).astype(np.float32)
    return y, res


def kernel(x, w_qkv, b_qkv, w_out, b_out):
    y, _ = run_device(x, w_qkv, b_qkv, w_out, b_out, trace=False)
    return y
